# revision 2
# baseline (speedup 1.0000x reference)
"""Trainium2 Bass kernel for a causal single-head attention block (v2).

Reference computation (per batch b):
    q = x @ Wq + bq ; k = x @ Wk + bk ; v = x @ Wv + bv      (x: [S, D])
    logits = q @ k.T  (causal masked), probs = softmax(logits / sqrt(128))
    out = concat([x, probs @ v], axis=-1)                     -> [S, D+v]

Shapes hardcoded: B=4, S=2048, D=1024, feature 128, 8 NeuronCores.

Sharding (SPMD): core c -> batch b = c//2, parity h = c%2.  Each core
computes the 8 query blocks at global positions {2j + h}, and K/V over the
full sequence of its batch.  x^T is shipped block-reordered (own parity
first) so the causal structure is identical across cores; mask values are
per-core DATA.

Schemes:
  - x^T row-major [p, t, col] but DMA'd in chunks that all have >=512-byte
    contiguous runs (full DMA-bus rate): the first 512 x-cols ship as four
    per-t-pair chunks (each with its wk slice in front, enabling the u-th
    DoubleRow projection pass the moment chunk u lands); the rest ships as
    full-t 512-col chunks.
  - All projections fp8 DoubleRow (weights x16 on host).
  - V projected directly in natural [s, f] layout (stationary = x^T block,
    moving = Wv): no transposes; evac'd in 4-block groups to vnat fp8
    (v*16, col 128 = 16.0 for the softmax denominators).
  - softmax exp: ONE affine op with uint8 saturating output whose bit
    pattern IS fp8e4m3(2^u) ~ exp (max rel err ~6%, on par with the fp8
    quantization already paid).  u8 = rint(psum * S8 + C8), on ACT or DVE
    (identical rint+saturate semantics verified on device) so the exp work
    is load-balanced across both engines.
  - logits are computed COLUMN-BLOCK-wise: for query block j and slot s,
    one PSUM holds all causal key blocks; PV j depends on just two exps.
  - causal mask: -1e5 added into the logits PSUM diag corner by a tiny
    matmul (identity stationary; tri / all-or-nothing per-core mask
    moving).  uint8 saturation then gives exact fp8 +0.
  - PV: fp8 DoubleRow over (m, m+8) block pairs (j+1 passes for query
    block j) accumulating [read | denom]; pairs of blocks are evac'd raw
    to SBUF, DMA'd out fp32, and NORMALIZED ON THE HOST (no recip/norm
    instructions on the critical chain).
  - bk drops (softmax shift); bv added on host; bq==0 fast path (the
    reference generates zeros), else applied on Q evac via ACT bias.
  - x passthrough half of the output is assembled on the HOST.
"""

import contextlib
import math

import numpy as np
import ml_dtypes

import concourse.bass as bass
import concourse.tile as tile
from concourse import bacc, mybir
from concourse.bass_utils import run_bass_kernel_spmd
from concourse.masks import make_identity

N_CORES = 8
B = 4
S = 2048
D = 1024
F = 128
NQT = 8
NKT = 16
QROWS = NQT * 128
SCALE = 1.0 / math.sqrt(F)
WSCALE = 16.0
LOG2E = 1.0 / math.log(2.0)
S8 = 8.0 * LOG2E * SCALE / (WSCALE * WSCALE)
C8 = 8.0 * (7.0 - 2.0 * LOG2E) - 0.25
MASKNEG = -1.0e5

FP32 = mybir.dt.float32
BF16 = mybir.dt.bfloat16
FP8 = mybir.dt.float8e4
U8 = mybir.dt.uint8
BF16_NP = ml_dtypes.bfloat16
FP8_NP = mybir.dt.np(FP8)
DR = mybir.MatmulPerfMode.DoubleRow

_compiled = {}

XO = 128  # wk cols in front of each t-row

# engine assignment tables (tuned against TimelineSim)
K_EVAC = ["act", "act", "act", "dve"]   # cols 0:512, B, C, D
Q_EVAC = ["dve", "dve"]                 # cols 0:512, B
V_EVAC = ["act", "dve", "dve", "act"]   # per 4-block group
PV_EVAC = ["dve", "act", "dve", "act"]  # per block pair


def _build(niter=1, bias_q=False):
    nc = bacc.Bacc("TRN2", target_bir_lowering=False, debug=False, num_devices=N_CORES)

    ABYTES = 128 * 2 * (XO + 512)
    BBYTES = 128 * 8 * 512
    xT_ext = nc.dram_tensor("xT", [4 * ABYTES + 3 * BBYTES], FP8,
                            kind="ExternalInput")
    wq_ext = nc.dram_tensor("wq", [128, 8, 128], FP8, kind="ExternalInput")
    wv_ext = nc.dram_tensor("wv", [128, 8, 128], FP8, kind="ExternalInput")
    bq_ext = (
        nc.dram_tensor("bq", [128, 1], FP32, kind="ExternalInput") if bias_q else None
    )
    mask_ext = nc.dram_tensor("masks", [128, 2, 128], BF16, kind="ExternalInput")
    out_ext = nc.dram_tensor("out", [4, 128, 2, 132], FP32, kind="ExternalOutput")

    with tile.TileContext(nc) as tc:
        with (
            tc.tile_pool(name="persist", bufs=1) as P,
            tc.tile_pool(name="ps_proj", bufs=2, space="PSUM") as ps_proj,
            tc.tile_pool(name="ps_log", bufs=2, space="PSUM") as ps_log,
            tc.tile_pool(name="ps_read", bufs=2, space="PSUM") as ps_read,
            tc.For_i(0, niter) if niter > 1 else contextlib.nullcontext(),
        ):
            xT_sb = P.tile([128, 8, XO + S], FP8)  # [d%128, t, wk|x col]
            wq_sb = P.tile([128, 8, 128], FP8)
            wv_sb = P.tile([128, 8, 128], FP8)
            bq_sb = P.tile([128, 1], FP32) if bias_q else None
            mask_sb = P.tile([128, 2, 128], BF16)  # [k, slot, q] maskneg
            ident = P.tile([128, 128], BF16)
            kT_sb = P.tile([128, S], BF16)
            qT_sb = P.tile([128, QROWS], BF16)
            vnat_sb = P.tile([128, NKT, 132], FP8)
            read_sb = P.tile([128, 4, 2, 132], FP32)
            expT_sb = P.tile([128, NKT, QROWS], U8)

            # ---- input DMAs: A0..A3 (t-pairs, wk + x cols 0:512), then
            # B, C, D (all t, 512 cols each) ----
            base = 0
            srcs = []
            for u in range(4):
                srcs.append(xT_ext[base:base + ABYTES].rearrange(
                    "(p t w) -> p t w", p=128, t=2))
                base += ABYTES
            for i in range(3):
                srcs.append(xT_ext[base:base + BBYTES].rearrange(
                    "(p t w) -> p t w", p=128, t=8))
                base += BBYTES

            def adma(eng, u):
                eng.dma_start(xT_sb[:, 2 * u:2 * u + 2, 0:XO + 512], srcs[u])

            def bdma(eng, i):
                o = XO + 512 * (i + 1)
                eng.dma_start(xT_sb[:, :, o:o + 512], srcs[4 + i])

            adma(nc.sync, 0)
            adma(nc.scalar, 1)
            adma(nc.sync, 2)
            adma(nc.scalar, 3)
            bdma(nc.sync, 0)
            bdma(nc.scalar, 1)
            bdma(nc.sync, 2)
            nc.gpsimd.dma_start(wq_sb[:], wq_ext[:])
            nc.gpsimd.dma_start(mask_sb[:], mask_ext[:])
            nc.gpsimd.dma_start(wv_sb[:], wv_ext[:])
            if bias_q:
                nc.gpsimd.dma_start(bq_sb[:], bq_ext[:])
            nc.vector.memset(vnat_sb[:, :, 128:129], WSCALE)
            make_identity(nc, ident[:])

            # ---- operand views (all inner-contiguous, row-major) ----
            def x_mov(u, s0, s1):
                return xT_sb[:, 2 * u:2 * u + 2, XO + s0:XO + s1]

            def x_stat(u, blk):
                o = XO + blk * 128
                return xT_sb[:, 2 * u:2 * u + 2, o:o + 128]

            def wk_stat(u):
                return xT_sb[:, 2 * u:2 * u + 2, 0:128]

            def wq_stat(u):
                return wq_sb[:, 2 * u:2 * u + 2, :]

            def wv_stat(u):
                return wv_sb[:, 2 * u:2 * u + 2, :]

            expT_pair = expT_sb[:].rearrange("p (two m) q -> p m two q", two=2)
            vnat_pair = vnat_sb[:].rearrange("p (two m) c -> p m two c", two=2)

            def evac(eng, dst, pp):
                if eng == "act":
                    nc.scalar.copy(dst, pp)
                else:
                    nc.vector.tensor_copy(dst, pp)

            # ---- projections ----
            def _evac_proj(dst_sb, sl, pp, eng):
                if dst_sb is qT_sb and bias_q:
                    nc.scalar.activation(
                        dst_sb[:, sl], pp[:],
                        mybir.ActivationFunctionType.Copy,
                        bias=bq_sb[:], scale=1.0,
                    )
                else:
                    evac(eng, dst_sb[:, sl], pp[:])

            def proj_chunk(stat_of, s0, dst_sb, eng):
                pp = ps_proj.tile([128, 512], FP32, tag="proj")
                for u in range(4):
                    nc.tensor.matmul(
                        pp[:], stat_of(u), x_mov(u, s0, s0 + 512),
                        start=(u == 0), stop=(u == 3), perf_mode=DR,
                    )
                _evac_proj(dst_sb, slice(s0, s0 + 512), pp, eng)

            def vproj4(g, eng):  # V natural for blocks 4g..4g+3
                pp = ps_proj.tile([128, 512], FP32, tag="proj")
                pv4 = pp[:].rearrange("p (four s) -> p four s", four=4)
                for q4 in range(4):
                    blk = 4 * g + q4
                    for u in range(4):
                        nc.tensor.matmul(
                            pv4[:, q4, :], x_stat(u, blk), wv_stat(u),
                            start=(u == 0), stop=(u == 3), perf_mode=DR,
                        )
                evac(eng, vnat_sb[:, 4 * g:4 * g + 4, 0:128], pp[:])

            # ---- column-block-wise logits + mask + exp ----
            def exp_op(eng, dst, src):
                if eng == "act":
                    nc.scalar.activation(
                        dst, src, mybir.ActivationFunctionType.Copy,
                        bias=C8, scale=S8,
                    )
                else:
                    nc.vector.tensor_scalar(
                        dst, src, S8, C8,
                        op0=mybir.AluOpType.mult, op1=mybir.AluOpType.add,
                    )

            _exp_rr = [0]

            def exp_auto():
                _exp_rr[0] += 1
                return "act" if _exp_rr[0] % 2 else "dve"

            def logits(slot, j, engs=None):
                qs = 128 * j
                n = j + 1
                pl = ps_log.tile([128, 1024], FP32, tag="log")
                for m in range(n):
                    nc.tensor.matmul(
                        pl[:, m * 128:(m + 1) * 128],
                        kT_sb[:, (8 * slot + m) * 128:(8 * slot + m) * 128 + 128],
                        qT_sb[:, qs:qs + 128],
                        start=True, stop=(m < j),
                    )
                nc.tensor.matmul(
                    pl[:, j * 128:n * 128], ident[:], mask_sb[:, slot, :],
                    start=False, stop=True, skip_group_check=True,
                )
                splits = [(0, n)] if j < 4 else [(0, 4), (4, n)]
                for a, b in splits:
                    eng = exp_auto() if engs is None else engs.pop(0)
                    exp_op(eng,
                           expT_sb[:, 8 * slot + a:8 * slot + b, qs:qs + 128],
                           pl[:, a * 128:b * 128])

            pv_tiles = {}

            def pv(j):
                if j % 2 == 0:
                    prt_new = ps_read.tile([128, 2, 132], FP32, tag="read")
                    pv_tiles[j // 2] = prt_new
                pr = pv_tiles[j // 2][:, j % 2, 0:129]
                jb = slice(j * 128, (j + 1) * 128)
                for m in range(j + 1):
                    nc.tensor.matmul(
                        pr,
                        expT_pair[:, m, :, jb].bitcast(FP8),
                        vnat_pair[:, m, :, 0:129].bitcast(FP8),
                        start=(m == 0), stop=(m == j),
                        perf_mode=DR,
                    )
                if j % 2:
                    evac(PV_EVAC[j // 2], read_sb[:, j // 2],
                         pv_tiles[j // 2][:])

            # ---- emission schedule ----
            out_dmas = []

            proj_chunk(wk_stat, 0, kT_sb, K_EVAC[0])
            proj_chunk(wq_stat, 0, qT_sb, Q_EVAC[0])
            vproj4(0, V_EVAC[0])
            proj_chunk(wk_stat, 512, kT_sb, K_EVAC[1])
            proj_chunk(wq_stat, 512, qT_sb, Q_EVAC[1])
            vproj4(1, V_EVAC[1])
            logits(0, 0)
            logits(0, 1)
            logits(0, 2)
            logits(0, 3)
            proj_chunk(wk_stat, 1024, kT_sb, K_EVAC[2])
            logits(0, 4)
            logits(0, 5)
            vproj4(2, V_EVAC[2])
            logits(0, 6)
            logits(0, 7)
            proj_chunk(wk_stat, 1536, kT_sb, K_EVAC[3])
            logits(1, 0)
            pv(0)
            logits(1, 1)
            pv(1)
            out_dmas.append(
                nc.gpsimd.dma_start(out=out_ext[0], in_=read_sb[:, 0]))
            logits(1, 2)
            pv(2)
            vproj4(3, V_EVAC[3])
            logits(1, 3)
            pv(3)
            out_dmas.append(
                nc.gpsimd.dma_start(out=out_ext[1], in_=read_sb[:, 1]))
            logits(1, 4)
            pv(4)
            logits(1, 5)
            pv(5)
            out_dmas.append(
                nc.sync.dma_start(out=out_ext[2], in_=read_sb[:, 2]))
            logits(1, 6)
            pv(6)
            logits(1, 7)
            pv(7)
            out_dmas.append(
                nc.scalar.dma_start(out=out_ext[3], in_=read_sb[:, 3]))

            if niter > 1:
                fence = nc.vector.memset(vnat_sb[:, 0, 129:130], 0.0)
                for d in out_dmas:
                    tile.add_dep_helper(fence.ins, d.ins, sync=True, reason="fence")

    nc.compile()
    return nc


def _get_compiled(niter=1, bias_q=False):
    key = f"nc{niter}b{int(bias_q)}"
    if key not in _compiled:
        _compiled[key] = _build(niter, bias_q=bias_q)
    return _compiled[key]


def _make_in_maps(inputs, Wq, bq, Wk, bk, Wv, bv):
    x = np.asarray(inputs, dtype=np.float32)
    assert x.shape == (B, S, D)

    def prep_w(w):
        w = (np.asarray(w, dtype=np.float32) * WSCALE).astype(FP8_NP)
        return np.ascontiguousarray(w.reshape(8, 128, 128).transpose(1, 0, 2))

    wq_np, wk_np, wv_np = prep_w(Wq), prep_w(Wk), prep_w(Wv)
    bias_q = bool(np.any(np.asarray(bq)))
    bq_np = (np.asarray(bq, np.float32) * WSCALE).reshape(128, 1)

    kk = np.arange(128)[:, None]
    qq = np.arange(128)[None, :]
    tri = np.where(qq >= kk, 0.0, MASKNEG).astype(np.float32)
    m_h = []
    for h in range(2):
        other = np.full((128, 128), 0.0 if h else MASKNEG, np.float32)
        m_h.append(np.ascontiguousarray(
            np.stack([tri, other], axis=1).astype(BF16_NP)))

    in_maps = []
    for c in range(N_CORES):
        b, h = divmod(c, 2)
        xb = x[b]
        order = np.concatenate([np.arange(h, NKT, 2), np.arange(1 - h, NKT, 2)])
        xb_local = xb.reshape(NKT, 128, D)[order].reshape(S, D)
        # row-major [p, t, s]
        xT_pts = np.ascontiguousarray(
            xb_local.reshape(S, 8, 128).transpose(2, 1, 0)).astype(FP8_NP)
        xT_mega = np.concatenate([wk_np, xT_pts], axis=2)  # [p, t, wk|s]
        parts = []
        for u in range(4):
            parts.append(
                np.ascontiguousarray(
                    xT_mega[:, 2 * u:2 * u + 2, 0:XO + 512]).reshape(-1))
        for i in range(3):
            o = XO + 512 * (i + 1)
            parts.append(
                np.ascontiguousarray(xT_mega[:, :, o:o + 512]).reshape(-1))
        xT = np.concatenate(parts)
        m = {"xT": xT, "wq": wq_np, "wv": wv_np, "masks": m_h[h]}
        if bias_q:
            m["bq"] = bq_np
        in_maps.append(m)
    return in_maps, bias_q


def _gather(results, x, bv):
    out = np.empty((B, S, D + F), dtype=np.float32)
    out[:, :, :D] = x
    bv = np.asarray(bv, np.float32)
    for c in range(N_CORES):
        b, h = divmod(c, 2)
        oc = np.asarray(results[c]["out"], dtype=np.float32)  # [4,128,2,132]
        for j in range(NQT):
            blk = oc[j // 2, :, j % 2, :]
            g = 2 * j + h
            out[b, g * 128:(g + 1) * 128, D:] = (
                blk[:, 0:128] / blk[:, 128:129] + bv)
    return out


def run(inputs, Wq, bq, Wk, bk, Wv, bv, trace=False):
    in_maps, bias_q = _make_in_maps(inputs, Wq, bq, Wk, bk, Wv, bv)
    nc = _get_compiled(bias_q=bias_q)
    x = np.asarray(inputs, dtype=np.float32)
    if trace:
        try:
            res = run_bass_kernel_spmd(nc, in_maps, list(range(N_CORES)), trace=True)
            return _gather(res.results, x, bv), res
        except Exception as e:
            print(f"trace run failed ({e!r}); falling back to untraced run")
    res = run_bass_kernel_spmd(nc, in_maps, list(range(N_CORES)))
    return _gather(res.results, x, bv), res


def kernel(inputs, Wq, bq, Wk, bk, Wv, bv):
    out, _ = run(inputs, Wq, bq, Wk, bk, Wv, bv, trace=False)
    return out


# revision 5
# speedup vs baseline: 1.2288x; 1.2288x over previous
"""Trainium2 Bass kernel for a causal single-head attention block (v2).

Reference computation (per batch b):
    q = x @ Wq + bq ; k = x @ Wk + bk ; v = x @ Wv + bv      (x: [S, D])
    logits = q @ k.T  (causal masked), probs = softmax(logits / sqrt(128))
    out = concat([x, probs @ v], axis=-1)                     -> [S, D+v]

Shapes hardcoded: B=4, S=2048, D=1024, feature 128, 8 NeuronCores.

Sharding (SPMD): core c -> batch b = c//2, parity h = c%2.  Each core
computes the 8 query blocks at global positions {2j + h}, and K/V over the
full sequence of its batch.  x^T is shipped block-reordered (own parity
first) so the causal structure is identical across cores; mask values are
per-core DATA.

Schemes:
  - x^T row-major [p, t, col] but DMA'd in chunks that all have >=512-byte
    contiguous runs (full DMA-bus rate): the first 512 x-cols ship as four
    per-t-pair chunks (each with its wk slice in front, enabling the u-th
    DoubleRow projection pass the moment chunk u lands); the rest ships as
    full-t 512-col chunks.
  - All projections fp8 DoubleRow (weights x16 on host).
  - V projected directly in natural [s, f] layout (stationary = x^T block,
    moving = Wv): no transposes; evac'd in 4-block groups to vnat fp8
    (v*16, col 128 = 16.0 for the softmax denominators).
  - softmax exp: ONE affine op with uint8 saturating output whose bit
    pattern IS fp8e4m3(2^u) ~ exp (max rel err ~6%, on par with the fp8
    quantization already paid).  u8 = rint(psum * S8 + C8), on ACT or DVE
    (identical rint+saturate semantics verified on device) so the exp work
    is load-balanced across both engines.
  - logits are computed COLUMN-BLOCK-wise: for query block j and slot s,
    one PSUM holds all causal key blocks; PV j depends on just two exps.
  - causal mask: -1e5 added into the logits PSUM diag corner by a tiny
    matmul (identity stationary; tri / all-or-nothing per-core mask
    moving).  uint8 saturation then gives exact fp8 +0.
  - PV: fp8 DoubleRow over (m, m+8) block pairs (j+1 passes for query
    block j) accumulating [read | denom]; pairs of blocks are evac'd raw
    to SBUF, DMA'd out fp32, and NORMALIZED ON THE HOST (no recip/norm
    instructions on the critical chain).
  - bk drops (softmax shift); bv added on host; bq==0 fast path (the
    reference generates zeros), else applied on Q evac via ACT bias.
  - x passthrough half of the output is assembled on the HOST.
"""

import contextlib
import math

import numpy as np
import ml_dtypes

import concourse.bass as bass
import concourse.tile as tile
from concourse import bacc, mybir
from concourse.bass_utils import run_bass_kernel_spmd
from concourse.masks import make_identity

N_CORES = 8
B = 4
S = 2048
D = 1024
F = 128
NQT = 8
NKT = 16
QROWS = NQT * 128
SCALE = 1.0 / math.sqrt(F)
WSCALE = 16.0
LOG2E = 1.0 / math.log(2.0)
S8 = 8.0 * LOG2E * SCALE / (WSCALE * WSCALE)
C8 = 8.0 * (7.0 - 2.0 * LOG2E) - 0.25
MASKNEG = -1.0e5

FP32 = mybir.dt.float32
BF16 = mybir.dt.bfloat16
FP8 = mybir.dt.float8e4
U8 = mybir.dt.uint8
BF16_NP = ml_dtypes.bfloat16
FP8_NP = mybir.dt.np(FP8)
DR = mybir.MatmulPerfMode.DoubleRow

_compiled = {}

XO = 128  # wk cols in front of each t-row

# engine assignment tables (tuned against TimelineSim)
K_EVAC = ["act", "act", "dve", "act"]   # cols 0:512, B, C, D
Q_EVAC = ["dve", "dve"]                 # cols 0:512, B
V_EVAC = ["act", "dve", "dve", "act"]   # per 4-block group
PV_EVAC = ["dve", "act", "dve", "act"]  # per block pair


def _build(niter=1, bias_q=False):
    nc = bacc.Bacc("TRN2", target_bir_lowering=False, debug=False, num_devices=N_CORES)

    ABYTES = 128 * 2 * (XO + 512)
    BBYTES = 128 * 8 * 512
    xT_ext = nc.dram_tensor("xT", [4 * ABYTES + 3 * BBYTES], FP8,
                            kind="ExternalInput")
    wq_ext = nc.dram_tensor("wq", [128, 8, 128], FP8, kind="ExternalInput")
    wv_ext = nc.dram_tensor("wv", [128, 8, 128], FP8, kind="ExternalInput")
    bq_ext = (
        nc.dram_tensor("bq", [128, 1], FP32, kind="ExternalInput") if bias_q else None
    )
    mask_ext = nc.dram_tensor("masks", [128, 2, 128], BF16, kind="ExternalInput")
    out_ext = nc.dram_tensor("out", [4, 128, 2, 132], FP32, kind="ExternalOutput")

    with tile.TileContext(nc) as tc:
        with (
            tc.tile_pool(name="persist", bufs=1) as P,
            tc.tile_pool(name="ps_proj", bufs=2, space="PSUM") as ps_proj,
            tc.tile_pool(name="ps_log", bufs=4, space="PSUM") as ps_log,
            tc.tile_pool(name="ps_read", bufs=2, space="PSUM") as ps_read,
            tc.For_i(0, niter) if niter > 1 else contextlib.nullcontext(),
        ):
            xT_sb = P.tile([128, 8, XO + S], FP8)  # [d%128, t, wk|x col]
            wq_sb = P.tile([128, 8, 128], FP8)
            wv_sb = P.tile([128, 8, 128], FP8)
            bq_sb = P.tile([128, 1], FP32) if bias_q else None
            mask_sb = P.tile([128, 2, 128], BF16)  # [k, slot, q] maskneg
            ident = P.tile([128, 128], BF16)
            kT_sb = P.tile([128, S], BF16)
            qT_sb = P.tile([128, QROWS], BF16)
            vnat_sb = P.tile([128, NKT, 132], FP8)
            read_sb = P.tile([128, 4, 2, 132], FP32)
            expT_sb = P.tile([128, NKT, QROWS], U8)

            # ---- input DMAs: A0..A3 (t-pairs, wk + x cols 0:512), then
            # B, C, D (all t, 512 cols each) ----
            base = 0
            srcs = []
            for u in range(4):
                srcs.append(xT_ext[base:base + ABYTES].rearrange(
                    "(p t w) -> p t w", p=128, t=2))
                base += ABYTES
            for i in range(3):
                srcs.append(xT_ext[base:base + BBYTES].rearrange(
                    "(p t w) -> p t w", p=128, t=8))
                base += BBYTES

            def adma(eng, u):
                eng.dma_start(xT_sb[:, 2 * u:2 * u + 2, 0:XO + 512], srcs[u])

            def bdma(eng, i):
                o = XO + 512 * (i + 1)
                eng.dma_start(xT_sb[:, :, o:o + 512], srcs[4 + i])

            adma(nc.sync, 0)
            adma(nc.scalar, 1)
            adma(nc.sync, 2)
            adma(nc.scalar, 3)
            bdma(nc.sync, 0)
            bdma(nc.scalar, 1)
            bdma(nc.sync, 2)
            nc.gpsimd.dma_start(wq_sb[:], wq_ext[:])
            nc.gpsimd.dma_start(mask_sb[:], mask_ext[:])
            nc.gpsimd.dma_start(wv_sb[:], wv_ext[:])
            if bias_q:
                nc.gpsimd.dma_start(bq_sb[:], bq_ext[:])
            nc.vector.memset(vnat_sb[:, :, 128:129], WSCALE)
            make_identity(nc, ident[:])

            # ---- operand views (all inner-contiguous, row-major) ----
            def x_mov(u, s0, s1):
                return xT_sb[:, 2 * u:2 * u + 2, XO + s0:XO + s1]

            def x_stat(u, blk):
                o = XO + blk * 128
                return xT_sb[:, 2 * u:2 * u + 2, o:o + 128]

            def wk_stat(u):
                return xT_sb[:, 2 * u:2 * u + 2, 0:128]

            def wq_stat(u):
                return wq_sb[:, 2 * u:2 * u + 2, :]

            def wv_stat(u):
                return wv_sb[:, 2 * u:2 * u + 2, :]

            expT_pair = expT_sb[:].rearrange("p (two m) q -> p m two q", two=2)
            vnat_pair = vnat_sb[:].rearrange("p (two m) c -> p m two c", two=2)

            def evac(eng, dst, pp):
                if eng == "act":
                    nc.scalar.copy(dst, pp)
                else:
                    nc.vector.tensor_copy(dst, pp)

            # ---- projections ----
            def _evac_proj(dst_sb, sl, pp, eng):
                if dst_sb is qT_sb and bias_q:
                    nc.scalar.activation(
                        dst_sb[:, sl], pp[:],
                        mybir.ActivationFunctionType.Copy,
                        bias=bq_sb[:], scale=1.0,
                    )
                else:
                    evac(eng, dst_sb[:, sl], pp[:])

            def proj_chunk(stat_of, s0, dst_sb, eng):
                pp = ps_proj.tile([128, 512], FP32, tag="proj")
                for u in range(4):
                    nc.tensor.matmul(
                        pp[:], stat_of(u), x_mov(u, s0, s0 + 512),
                        start=(u == 0), stop=(u == 3), perf_mode=DR,
                    )
                _evac_proj(dst_sb, slice(s0, s0 + 512), pp, eng)

            def vproj4(g, eng, pool=None):  # V natural, blocks 4g..4g+3
                pp = (pool or ps_proj).tile([128, 512], FP32, tag="proj"
                                            if pool is None else "log")
                pv4 = pp[:].rearrange("p (four s) -> p four s", four=4)
                for q4 in range(4):
                    blk = 4 * g + q4
                    for u in range(4):
                        nc.tensor.matmul(
                            pv4[:, q4, :], x_stat(u, blk), wv_stat(u),
                            start=(u == 0), stop=(u == 3), perf_mode=DR,
                        )
                evac(eng, vnat_sb[:, 4 * g:4 * g + 4, 0:128], pp[:])

            # ---- column-block-wise logits + mask + exp ----
            def exp_op(eng, dst, src):
                if eng == "act":
                    nc.scalar.activation(
                        dst, src, mybir.ActivationFunctionType.Copy,
                        bias=C8, scale=S8,
                    )
                else:
                    nc.vector.tensor_scalar(
                        dst, src, S8, C8,
                        op0=mybir.AluOpType.mult, op1=mybir.AluOpType.add,
                    )

            _exp_rr = [0]

            def exp_auto():
                _exp_rr[0] += 1
                return "act" if _exp_rr[0] % 2 else "dve"

            def logits(slot, j, engs=None):
                # sub-units of <= 4 key blocks, each with its own 1-bank
                # PSUM and one exp -> 4-deep pipelining, short latency
                qs = 128 * j
                n = j + 1
                for a, b in ([(0, n)] if j < 4 else [(0, 4), (4, n)]):
                    pl = ps_log.tile([128, 512], FP32, tag="log")
                    for m in range(a, b):
                        nc.tensor.matmul(
                            pl[:, (m - a) * 128:(m - a + 1) * 128],
                            kT_sb[:, (8 * slot + m) * 128:
                                  (8 * slot + m) * 128 + 128],
                            qT_sb[:, qs:qs + 128],
                            start=True, stop=(m < j),
                        )
                    if a <= j < b:
                        nc.tensor.matmul(
                            pl[:, (j - a) * 128:(j - a + 1) * 128],
                            ident[:], mask_sb[:, slot, :],
                            start=False, stop=True, skip_group_check=True,
                        )
                    eng = exp_auto() if engs is None else engs.pop(0)
                    exp_op(eng,
                           expT_sb[:, 8 * slot + a:8 * slot + b, qs:qs + 128],
                           pl[:, 0:(b - a) * 128])

            pv_tiles = {}

            def pv(j, split_evac=False):
                if j % 2 == 0:
                    prt_new = ps_read.tile([128, 2, 132], FP32, tag="read")
                    pv_tiles[j // 2] = prt_new
                pr = pv_tiles[j // 2][:, j % 2, 0:129]
                jb = slice(j * 128, (j + 1) * 128)
                for m in range(j + 1):
                    nc.tensor.matmul(
                        pr,
                        expT_pair[:, m, :, jb].bitcast(FP8),
                        vnat_pair[:, m, :, 0:129].bitcast(FP8),
                        start=(m == 0), stop=(m == j),
                        perf_mode=DR,
                    )
                if split_evac:
                    # evac this block alone (shortens the final chain)
                    evac(PV_EVAC[j // 2], read_sb[:, j // 2, j % 2],
                         pv_tiles[j // 2][:, j % 2])
                elif j % 2:
                    evac(PV_EVAC[j // 2], read_sb[:, j // 2],
                         pv_tiles[j // 2][:])

            # ---- emission schedule ----
            out_dmas = []

            # A-phase: K/Q/V passes interleaved per A chunk (pass u runs
            # the moment chunk u lands), then own-slot units j 0..3
            ppk = ps_proj.tile([128, 512], FP32, tag="proj")
            ppq = ps_proj.tile([128, 512], FP32, tag="proj")
            for u in range(4):
                nc.tensor.matmul(
                    ppk[:], wk_stat(u), x_mov(u, 0, 512),
                    start=(u == 0), stop=(u == 3), perf_mode=DR,
                )
                nc.tensor.matmul(
                    ppq[:], wq_stat(u), x_mov(u, 0, 512),
                    start=(u == 0), stop=(u == 3), perf_mode=DR,
                )
            _evac_proj(kT_sb, slice(0, 512), ppk, K_EVAC[0])
            _evac_proj(qT_sb, slice(0, 512), ppq, Q_EVAC[0])
            vproj4(0, V_EVAC[0], pool=ps_log)
            logits(0, 0)
            logits(0, 1)
            proj_chunk(wk_stat, 512, kT_sb, K_EVAC[1])
            logits(0, 2)
            proj_chunk(wq_stat, 512, qT_sb, Q_EVAC[1])
            logits(0, 3)
            vproj4(1, V_EVAC[1])
            # B-phase: own-slot units j 4..7
            logits(0, 4)
            logits(0, 5)
            proj_chunk(wk_stat, 1024, kT_sb, K_EVAC[2])
            logits(0, 6)
            logits(0, 7)
            vproj4(2, V_EVAC[2])
            # C-phase: slot-1 units j 0..3 + PV
            logits(1, 0)
            proj_chunk(wk_stat, 1536, kT_sb, K_EVAC[3])
            pv(0)
            logits(1, 1)
            pv(1)
            out_dmas.append(
                nc.gpsimd.dma_start(out=out_ext[0], in_=read_sb[:, 0]))
            logits(1, 2)
            vproj4(3, V_EVAC[3])
            pv(2)
            logits(1, 3)
            pv(3)
            out_dmas.append(
                nc.gpsimd.dma_start(out=out_ext[1], in_=read_sb[:, 1]))
            # D-phase: slot-1 units j 4..7 + PV
            logits(1, 4)
            pv(4)
            logits(1, 5)
            pv(5)
            out_dmas.append(
                nc.sync.dma_start(out=out_ext[2], in_=read_sb[:, 2]))
            logits(1, 6)
            pv(6, split_evac=True)
            out_dmas.append(
                nc.sync.dma_start(out=out_ext[3, :, 0:1],
                                  in_=read_sb[:, 3, 0:1]))
            logits(1, 7)
            pv(7, split_evac=True)
            out_dmas.append(
                nc.scalar.dma_start(out=out_ext[3, :, 1:2],
                                    in_=read_sb[:, 3, 1:2]))

            if niter > 1:
                fence = nc.vector.memset(vnat_sb[:, 0, 129:130], 0.0)
                for d in out_dmas:
                    tile.add_dep_helper(fence.ins, d.ins, sync=True, reason="fence")

    nc.compile()
    return nc


def _get_compiled(niter=1, bias_q=False):
    key = f"nc{niter}b{int(bias_q)}"
    if key not in _compiled:
        _compiled[key] = _build(niter, bias_q=bias_q)
    return _compiled[key]


def _make_in_maps(inputs, Wq, bq, Wk, bk, Wv, bv):
    x = np.asarray(inputs, dtype=np.float32)
    assert x.shape == (B, S, D)

    def prep_w(w):
        w = (np.asarray(w, dtype=np.float32) * WSCALE).astype(FP8_NP)
        return np.ascontiguousarray(w.reshape(8, 128, 128).transpose(1, 0, 2))

    wq_np, wk_np, wv_np = prep_w(Wq), prep_w(Wk), prep_w(Wv)
    bias_q = bool(np.any(np.asarray(bq)))
    bq_np = (np.asarray(bq, np.float32) * WSCALE).reshape(128, 1)

    kk = np.arange(128)[:, None]
    qq = np.arange(128)[None, :]
    tri = np.where(qq >= kk, 0.0, MASKNEG).astype(np.float32)
    m_h = []
    for h in range(2):
        other = np.full((128, 128), 0.0 if h else MASKNEG, np.float32)
        m_h.append(np.ascontiguousarray(
            np.stack([tri, other], axis=1).astype(BF16_NP)))

    in_maps = []
    for c in range(N_CORES):
        b, h = divmod(c, 2)
        xb = x[b]
        order = np.concatenate([np.arange(h, NKT, 2), np.arange(1 - h, NKT, 2)])
        xb_local = xb.reshape(NKT, 128, D)[order].reshape(S, D)
        # row-major [p, t, s]
        xT_pts = np.ascontiguousarray(
            xb_local.reshape(S, 8, 128).transpose(2, 1, 0)).astype(FP8_NP)
        xT_mega = np.concatenate([wk_np, xT_pts], axis=2)  # [p, t, wk|s]
        parts = []
        for u in range(4):
            parts.append(
                np.ascontiguousarray(
                    xT_mega[:, 2 * u:2 * u + 2, 0:XO + 512]).reshape(-1))
        for i in range(3):
            o = XO + 512 * (i + 1)
            parts.append(
                np.ascontiguousarray(xT_mega[:, :, o:o + 512]).reshape(-1))
        xT = np.concatenate(parts)
        m = {"xT": xT, "wq": wq_np, "wv": wv_np, "masks": m_h[h]}
        if bias_q:
            m["bq"] = bq_np
        in_maps.append(m)
    return in_maps, bias_q


def _gather(results, x, bv):
    out = np.empty((B, S, D + F), dtype=np.float32)
    out[:, :, :D] = x
    bv = np.asarray(bv, np.float32)
    for c in range(N_CORES):
        b, h = divmod(c, 2)
        oc = np.asarray(results[c]["out"], dtype=np.float32)  # [4,128,2,132]
        for j in range(NQT):
            blk = oc[j // 2, :, j % 2, :]
            g = 2 * j + h
            out[b, g * 128:(g + 1) * 128, D:] = (
                blk[:, 0:128] / blk[:, 128:129] + bv)
    return out


def run(inputs, Wq, bq, Wk, bk, Wv, bv, trace=False):
    in_maps, bias_q = _make_in_maps(inputs, Wq, bq, Wk, bk, Wv, bv)
    nc = _get_compiled(bias_q=bias_q)
    x = np.asarray(inputs, dtype=np.float32)
    if trace:
        try:
            res = run_bass_kernel_spmd(nc, in_maps, list(range(N_CORES)), trace=True)
            return _gather(res.results, x, bv), res
        except Exception as e:
            print(f"trace run failed ({e!r}); falling back to untraced run")
    res = run_bass_kernel_spmd(nc, in_maps, list(range(N_CORES)))
    return _gather(res.results, x, bv), res


def kernel(inputs, Wq, bq, Wk, bk, Wv, bv):
    out, _ = run(inputs, Wq, bq, Wk, bk, Wv, bv, trace=False)
    return out
